# revision 1
# baseline (speedup 1.0000x reference)
"""Bass/Tile TRN2 kernel for CrossAttention (B=2, N=4096, D=512, H=8, DH=64).

Sharding: batch*heads over 8 cores — core c handles batch c//4 and heads
(c%4)*2, (c%4)*2+1. Each core computes its two heads' attention and the
partial output projection O_h @ Wo_h; the host sums the 4 partials per batch.

Per-core dataflow (one NeuronCore, Tile-scheduled; per-block tiles so the
projection phase overlaps the attention phase):
  xT [512,4096] (host-pretransposed x[b]) -> SBUF per 512-column block
  Qt,Kt [128,512] per block = W^T x^T   (heads stacked: h0 = partitions 0:64)
  V natural [128,130] per 128-row j-chunk as [V_h0 | 1 | V_h1 | 1]
  per (i-block 512, j-chunk 128):
     St = Kt^T Qt                  (row-packed 2 heads -> one 2-bank PSUM tile)
     Pt = exp(SCALE*St)            (ScalarE [128,1024] call; no max-sub:
                                    logits are O(1) for this problem family)
     O' += [V|1]^T Pt              (PSUM accum; row 64 = softmax denominator)
  epilogue: rinv = 1/O'[64] (DVE), transpose rinv to partitions via K=1 fp32
  matmul against ones, project unnormalized O with Wo (row-packed), scale the
  two head partials by rinv as per-partition scalars, sum + bias, DMA out.
"""

import sys

if "/opt/trn_rl_repo" not in sys.path:
    sys.path.insert(0, "/opt/trn_rl_repo")

import numpy as np

B, N, D = 2, 4096, 512
H, DH = 8, 64
SCALE = DH ** -0.5
P = 128
IB = 512            # i/column block
NDC = D // P        # 4 contraction chunks for projections
NIB = N // IB       # 8
NJC = N // P        # 32 key chunks
NQ = IB // P        # 4 out-proj chunks per i-block

_CACHE: dict = {}


def _build(n_attn_ib=NIB):
    import concourse.mybir as mybir
    from concourse import bacc
    from concourse.tile import TileContext

    f32 = mybir.dt.float32
    f32r = mybir.dt.float32r
    Exp = mybir.ActivationFunctionType.Exp

    nc = bacc.Bacc("TRN2")
    xT = nc.dram_tensor("xT", [D, N], f32r, kind="ExternalInput")
    wq = nc.dram_tensor("wq", [D, 2 * DH], f32r, kind="ExternalInput")
    wk = nc.dram_tensor("wk", [D, 2 * DH], f32r, kind="ExternalInput")
    wv = nc.dram_tensor("wv", [D, 2 * DH], f32r, kind="ExternalInput")
    wo = nc.dram_tensor("wo", [2 * DH, D], f32r, kind="ExternalInput")
    bo = nc.dram_tensor("bo", [D], f32, kind="ExternalInput")
    out = nc.dram_tensor("out", [N, D], f32, kind="ExternalOutput")

    with TileContext(nc) as tc, \
         tc.tile_pool(name="persist", bufs=1) as pp:
        # per-block persistent SBUF tensors (separate tiles => fine deps)
        xtb = [pp.tile([P, NDC, IB], f32r, name=f"xt{i}", tag=f"xt{i}")
               for i in range(NIB)]
        qtb = [pp.tile([P, IB], f32r, name=f"qt{i}", tag=f"qt{i}")
               for i in range(NIB)]
        ktb = [pp.tile([P, IB], f32r, name=f"kt{i}", tag=f"kt{i}")
               for i in range(NIB)]
        vtb = [pp.tile([P, NQ, 130], f32r, name=f"vt{i}", tag=f"vt{i}")
               for i in range(NIB)]
        wq_sb = pp.tile([P, NDC, 2 * DH], f32r, name="wq_sb", tag="wq")
        wk_sb = pp.tile([P, NDC, 2 * DH], f32r, name="wk_sb", tag="wk")
        # wv padded with wq columns to a 256-wide moving operand: fp32r
        # matmuls only hit full rate at free dim >= 256 (cols 128:256 unused)
        wv_sb = pp.tile([P, NDC, 4 * DH], f32r, name="wv_sb", tag="wv")
        wo_sb = pp.tile([P, D], f32r, name="wo_sb", tag="wo")
        bo_sb = pp.tile([1, D], f32, name="bo_sb", tag="bos")
        bo_bc = pp.tile([P, D], f32, name="bo_bc", tag="bob")
        one_sb = pp.tile([1, 1], f32, name="one_sb", tag="one")

        for dc in range(NDC):
            nc.sync.dma_start(wq_sb[:, dc, :], wq[dc * P:(dc + 1) * P, :])
            nc.sync.dma_start(wk_sb[:, dc, :], wk[dc * P:(dc + 1) * P, :])
            nc.sync.dma_start(wv_sb[:, dc, 0:2 * DH],
                              wv[dc * P:(dc + 1) * P, :])
            nc.sync.dma_start(wv_sb[:, dc, 2 * DH:4 * DH],
                              wq[dc * P:(dc + 1) * P, :])
        nc.sync.dma_start(wo_sb[:], wo[:, :])
        nc.sync.dma_start(bo_sb[:], bo[None, :])
        nc.gpsimd.partition_broadcast(bo_bc[:], bo_sb[:])
        nc.vector.memset(one_sb[:], 1.0)
        for ibb in range(NIB):
            nc.vector.memset(vtb[ibb][:, :, 64:65].bitcast(f32), 1.0)
            nc.vector.memset(vtb[ibb][:, :, 129:130].bitcast(f32), 1.0)

        with tc.tile_pool(name="ps", bufs=2, space="PSUM") as ps_pool, \
             tc.tile_pool(name="po", bufs=2, space="PSUM") as po_pool, \
             tc.tile_pool(name="pe", bufs=1, space="PSUM") as pe_pool, \
             tc.tile_pool(name="pt", bufs=6) as pt_pool, \
             tc.tile_pool(name="ep", bufs=3) as ep_pool, \
             tc.tile_pool(name="ot", bufs=6) as ot_pool:

            for ibb in range(NIB):
                for dc in range(NDC):
                    nc.sync.dma_start(xtb[ibb][:, dc, :],
                                      xT[dc * P:(dc + 1) * P,
                                         ibb * IB:(ibb + 1) * IB])

            def phase_a_block(ibb):
                """Project column block ibb's K, V, Q."""
                xt = xtb[ibb]
                for dst, w_sb in ((ktb[ibb], wk_sb), (qtb[ibb], wq_sb)):
                    pq = ps_pool.tile([P, IB], f32, tag="st", name="pq")
                    for dc in range(NDC):
                        nc.tensor.matmul(pq[:], w_sb[:, dc, :], xt[:, dc, :],
                                         start=(dc == 0), stop=(dc == NDC - 1))
                    nc.vector.tensor_copy(dst[:], pq[:])
                for q in range(NQ):
                    pv = ps_pool.tile([P, 4 * DH], f32, tag="st", name="pv")
                    for dc in range(NDC):
                        nc.tensor.matmul(
                            pv[:], xt[:, dc, q * P:(q + 1) * P],
                            wv_sb[:, dc, :],
                            start=(dc == 0), stop=(dc == NDC - 1))
                    nc.vector.tensor_copy(vtb[ibb][:, q, 0:DH], pv[:, 0:DH])
                    nc.vector.tensor_copy(vtb[ibb][:, q, 65:65 + DH],
                                          pv[:, DH:2 * DH])

            # ---- attention (phase A for block b fused before its first use) ----
            for ib in range(n_attn_ib):
                qt = qtb[ib]
                o0 = po_pool.tile([65, IB], f32, tag="o", name="o0")
                o1 = po_pool.tile([65, IB], f32, tag="o", name="o1")
                for jc in range(NJC):
                    if ib == 0 and jc % NQ == 0:
                        phase_a_block(jc // NQ)
                    kt = ktb[jc // NQ]
                    k0 = (jc % NQ) * P
                    st = ps_pool.tile([P, 2 * IB], f32, tag="st", name="st")
                    nc.tensor.matmul(st[:, 0:IB],
                                     kt[0:DH, k0:k0 + P], qt[0:DH, :],
                                     start=True, stop=True,
                                     tile_position=(0, 0))
                    nc.tensor.matmul(st[:, IB:2 * IB],
                                     kt[DH:P, k0:k0 + P], qt[DH:P, :],
                                     start=True, stop=True,
                                     tile_position=(64, 0))
                    pt = pt_pool.tile([P, 2 * IB], f32r, tag="pt", name="pt")
                    nc.scalar.activation(pt[:], st[:], Exp, scale=SCALE)
                    vt = vtb[jc // NQ][:, jc % NQ, :]
                    nc.tensor.matmul(o0[:], vt[:, 0:65], pt[:, 0:IB],
                                     start=(jc == 0), stop=(jc == NJC - 1))
                    nc.tensor.matmul(o1[:], vt[:, 65:130], pt[:, IB:2 * IB],
                                     start=(jc == 0), stop=(jc == NJC - 1))
                # epilogue: denominators -> per-partition scalars via K=1 fp32
                # matmul transpose; project unnormalized; scale, sum, bias.
                rinv0 = ep_pool.tile([1, IB], f32, tag="rinv", name="rinv0")
                rinv1 = ep_pool.tile([1, IB], f32, tag="rinv", name="rinv1")
                nc.vector.reciprocal(rinv0[:], o0[64:65, :])
                nc.vector.reciprocal(rinv1[:], o1[64:65, :])
                rtp = pe_pool.tile([P, 2 * NQ], f32, tag="ep", name="rtp")
                for q in range(NQ):
                    nc.tensor.matmul(rtp[:, q:q + 1],
                                     rinv0[0:1, q * P:(q + 1) * P], one_sb[:],
                                     start=True, stop=True)
                    nc.tensor.matmul(rtp[:, NQ + q:NQ + q + 1],
                                     rinv1[0:1, q * P:(q + 1) * P], one_sb[:],
                                     start=True, stop=True)
                rts = ep_pool.tile([P, 2 * NQ], f32, tag="rts", name="rts")
                nc.vector.tensor_copy(rts[:], rtp[:])
                ots = ep_pool.tile([P, IB], f32r, tag="otn", name="ots")
                nc.vector.tensor_copy(ots[0:DH, :], o0[0:DH, :])
                nc.vector.tensor_copy(ots[DH:P, :], o1[0:DH, :])
                for q in range(NQ):
                    q0, q1 = q * P, (q + 1) * P
                    ppx = pe_pool.tile([P, 2 * D], f32, tag="ep", name="ppx")
                    nc.tensor.matmul(ppx[:, 0:D], ots[0:DH, q0:q1],
                                     wo_sb[0:DH, :],
                                     start=True, stop=True,
                                     tile_position=(0, 0))
                    nc.tensor.matmul(ppx[:, D:2 * D], ots[DH:P, q0:q1],
                                     wo_sb[DH:P, :],
                                     start=True, stop=True,
                                     tile_position=(64, 0))
                    t0 = ot_pool.tile([P, D], f32, tag="t0", name="t0")
                    t1 = ot_pool.tile([P, D], f32, tag="t1", name="t1")
                    nc.vector.tensor_scalar_mul(t0[:], ppx[:, 0:D],
                                                rts[:, q:q + 1])
                    nc.vector.tensor_scalar_mul(t1[:], ppx[:, D:2 * D],
                                                rts[:, NQ + q:NQ + q + 1])
                    otile = ot_pool.tile([P, D], f32, tag="out", name="otile")
                    nc.gpsimd.tensor_add(otile[:], t0[:], t1[:])
                    nc.gpsimd.tensor_add(otile[:], otile[:], bo_bc[:])
                    nc.sync.dma_start(out[ib * IB + q0:ib * IB + q1, :],
                                      otile[:])

    nc.compile()
    return nc


def _get_nc():
    if "nc" not in _CACHE:
        _CACHE["nc"] = _build()
    return _CACHE["nc"]


def kernel(x, Wq, Wk, Wv, Wo, bo):
    from concourse.bass_utils import run_bass_kernel_spmd

    x = np.asarray(x, dtype=np.float32)
    Wq = np.asarray(Wq, dtype=np.float32)
    Wk = np.asarray(Wk, dtype=np.float32)
    Wv = np.asarray(Wv, dtype=np.float32)
    Wo = np.asarray(Wo, dtype=np.float32)
    bo = np.asarray(bo, dtype=np.float32)

    nc = _get_nc()

    xTs = [np.ascontiguousarray(x[b].T) for b in range(B)]
    zeros_bo = np.zeros_like(bo)
    in_maps = []
    for c in range(8):
        b, p = c // 4, c % 4
        sl = slice(p * 2 * DH, (p + 1) * 2 * DH)
        in_maps.append({
            "xT": xTs[b],
            "wq": np.ascontiguousarray(Wq[:, sl]),
            "wk": np.ascontiguousarray(Wk[:, sl]),
            "wv": np.ascontiguousarray(Wv[:, sl]),
            "wo": np.ascontiguousarray(Wo[sl, :]),
            "bo": bo if p == 0 else zeros_bo,
        })

    try:
        res = run_bass_kernel_spmd(nc, in_maps, core_ids=list(range(8)))
    except Exception:
        # transient device wedge (NRT_EXEC_UNIT_UNRECOVERABLE) — retry once
        import time as _time
        _time.sleep(45)
        res = run_bass_kernel_spmd(nc, in_maps, core_ids=list(range(8)))
    parts = [res.results[c]["out"] for c in range(8)]
    full = np.stack([
        parts[0] + parts[1] + parts[2] + parts[3],
        parts[4] + parts[5] + parts[6] + parts[7],
    ]).astype(np.float32)
    return full

